# revision 73
# baseline (speedup 1.0000x reference)
"""Trainium2 Bass kernel for nn_MergeMetaCNN (hypernetwork MLP -> grouped conv -> CNN).

Data-parallel over batch: 32 samples -> 8 NeuronCores, 4 samples each.

Per-core pipeline (all math on device):
  1. MLP (fp32 matmuls) -> per-sample conv1 filters + biases (as raw^T in DRAM).
  2. conv1 (grouped 3x3) as block-diag matmul: stationary [108, 32] bf16
     (rows = (tap, sample, cin)), moving operand = replicated/shifted bf16 X
     band tiles [108, 34*226] built by ONE DMA per band from the padded
     bf16 X planes (each partition = one (dy, dx, plane) shifted window).
     Epilogue (ACT/DVE alternating) writes y rows into padded y planes.
  3. y replicas: one DMA per band copies the y planes into Y4
     [128 = (s, t, dy'), PLANE] with per-partition row shift dy' in {0..3}.
  4. conv2 (8->64, 3x3) with vertical pixel-pairing: M = 128 = (pix in {0,1},
     cout); contraction K = 32 = (t, dy') per sample (PE row-group s via
     tile_position); the 3 kernel columns accumulate in PSUM as 3 matmuls
     whose rhs APs read Y4 at free-dim offsets dx in {0,1,2}.
  5. Epilogue relu(x + b) split across ScalarE/VectorE into bf16 staging;
     one SWDGE store DMA per band writes an even/odd row-block layout
     (contiguous 7KB runs); host un-interleaves + upcasts to fp32.
  conv1 band g+2 / conv2 band g are interleaved in program order so the
  xr gather, conv1 MM+epilogue, yb replication, conv2 MM+epilogue and the
  store all pipeline across bands.
"""

import numpy as np
import ml_dtypes
from contextlib import ExitStack

import concourse.bass as bass
import concourse.tile as tile
from concourse import bacc, mybir
from concourse.bass_utils import run_bass_kernel_spmd

AP = bass.AP
f32 = mybir.dt.float32
bf16 = mybir.dt.bfloat16
AF = mybir.ActivationFunctionType
ALU = mybir.AluOpType

# Problem constants (hardcoded per contract)
B, CIN, H, W = 32, 3, 224, 224
TMP, K, FLAT, COUT = 8, 3, 128, 64
MLP_OUT = TMP * CIN * K * K + TMP  # 224
META = 0.1
NCORES = 8
SPC = B // NCORES                  # 4 samples per core
PH, PW = H + 2, W + 2              # 226 (zero-pad 1 on each side)
PLANE = PH * PW                    # 51076
PP4 = 230 * PW                     # X plane stride: 1 extra lead row + tail
K27 = CIN * K * K                  # 27
K108 = SPC * K27                   # conv1 contraction (block-diag 4 samples)
NB32 = H // 32                     # 7 bands of 32 rows
XB4 = 37 * PW + W                  # staged X band (exact conv1 read extent)
YBTS = 37 * PW                     # y band tile stride (36 rows + 1 overread)
YB2 = 34 * PW                      # conv2 y-replica band length
HWP = H * W
HB = (H // 2) * W                  # even/odd half-plane block (112*224)

_CACHE = {}


def build_module(repeat=1, loop_n=None, variant="full", store_eng="gpsimd",
                 inner=1, yb_eng="split", xb_split=False, store_split=True,
                 phased=False):
    key = ("nc", repeat, loop_n, variant, store_eng, inner, yb_eng,
           xb_split, store_split, phased)
    if key in _CACHE:
        return _CACHE[key]
    nc = bacc.Bacc("TRN2", target_bir_lowering=False, debug=False, num_devices=NCORES)

    # ---- DRAM I/O (per-core shapes) ----
    # padXb3: 36 planes (s, ci, dx) of bf16 padded X, pre-shifted by dx on
    # the host; each plane has 1 extra zero lead row + zero tail rows so
    # every band reads a uniform in-bounds window
    padXb3 = nc.dram_tensor("padXb3", [3 * SPC * CIN, PP4], bf16,
                            kind="ExternalInput")
    fxT = nc.dram_tensor("fxT", [FLAT, SPC], f32, kind="ExternalInput")
    W1 = nc.dram_tensor("W1", [FLAT, MLP_OUT], f32, kind="ExternalInput")
    b1 = nc.dram_tensor("b1", [MLP_OUT], f32, kind="ExternalInput")
    # W2P: columns 0..215 permuted to q = ci*72 + (dy*3+dx)*8 + t; row 224 = b2
    W2P = nc.dram_tensor("W2P", [MLP_OUT + 1, MLP_OUT], f32, kind="ExternalInput")
    # lhsT1z: zeros [108, 32] staging for the conv1-stationary scatter
    lhsT1z = nc.dram_tensor("lhsT1z", [K108, SPC * TMP], f32,
                            kind="ExternalInput")
    bias_d = nc.dram_tensor("bias_d", [SPC * TMP], f32)   # conv1 bias staging
    cnn_wP = nc.dram_tensor("cnn_wP", [128, 3 * 128], f32, kind="ExternalInput")
    cnn_b128 = nc.dram_tensor("cnn_b128", [128], f32, kind="ExternalInput")
    # timing builds (loop_n) keep the result in DRAM and return a tiny dummy
    # output so runs don't ship 25.7MB/core back through the axon tunnel
    timing = loop_n is not None
    # out layout: [pix, s, c, 112, 224] -- even rows block then odd rows
    # block (host un-interleaves); gives 7168B-contiguous store runs
    out = nc.dram_tensor("out", [2, SPC * COUT * HB], bf16,
                         kind="Internal" if timing else "ExternalOutput")
    if timing:
        dummy = nc.dram_tensor("tdummy", [1, 32], f32, kind="ExternalOutput")

    with tile.TileContext(nc) as tc, ExitStack() as ctx:
        cpool = ctx.enter_context(tc.tile_pool(name="consts", bufs=1))
        spool = ctx.enter_context(tc.tile_pool(name="stageA", bufs=1))
        mlp_ctx = ExitStack()
        mpsum = mlp_ctx.enter_context(tc.tile_pool(name="mlp_psum", bufs=2, space="PSUM"))

        # ================= Stage A: MLP + weight prep =================
        w1sb = cpool.tile([FLAT, MLP_OUT], f32)
        nc.sync.dma_start(w1sb[:], W1.ap())
        w2a = cpool.tile([128, MLP_OUT], f32)
        nc.sync.dma_start(w2a[:], W2P.ap()[0:128, :])
        w2b = cpool.tile([97, MLP_OUT], f32)          # rows 128..224 (incl. b2)
        nc.sync.dma_start(w2b[:], W2P.ap()[128:225, :])
        fx_sb = cpool.tile([FLAT, SPC], f32)
        nc.sync.dma_start(fx_sb[:], fxT.ap())
        b1a = cpool.tile([128, 1], f32)
        nc.sync.dma_start(b1a[:], b1.ap()[0:128].unsqueeze(1))
        b1b = cpool.tile([96, 1], f32)
        nc.sync.dma_start(b1b[:], b1.ap()[128:224].unsqueeze(1))
        cnnb_sb = cpool.tile([128, 1], f32)
        nc.sync.dma_start(cnnb_sb[:], cnn_b128.ap().unsqueeze(1))
        lhsTc = cpool.tile([128, 3 * 128], bf16)
        nc.gpsimd.dma_start(lhsTc[:], cnn_wP.ap())  # cast f32 -> bf16

        WSCALE = META / K27
        if timing:
            nc.sync.dma_start(dummy.ap(), b1.ap()[0:32].unsqueeze(0))

        # hid^T = relu(W1^T @ fxT + b1)  [224, SPC]; extra ones-row for b2
        ph_a = mpsum.tile([128, SPC], f32, tag="mp")
        nc.tensor.matmul(ph_a[:], lhsT=w1sb[:, 0:128], rhs=fx_sb[:], start=True, stop=True)
        hida = spool.tile([128, SPC], f32)
        nc.scalar.activation(hida[:], ph_a[:], func=AF.Relu, bias=b1a[:])
        ph_b = mpsum.tile([96, SPC], f32, tag="mp")
        nc.tensor.matmul(ph_b[:], lhsT=w1sb[:, 128:224], rhs=fx_sb[:], start=True, stop=True)
        hidb = spool.tile([97, SPC], f32)
        nc.vector.memset(hidb[96:97, :], 1.0)
        nc.scalar.activation(hidb[0:96, :], ph_b[:], func=AF.Relu, bias=b1b[:])

        # raw = (hid^T)^T @ W2P * WSCALE  [SPC, 224]  (sample-major, permuted)
        praw = mpsum.tile([SPC, MLP_OUT], f32, tag="mp")
        nc.tensor.matmul(praw[:], lhsT=hida[:], rhs=w2a[:], start=True, stop=False)
        nc.tensor.matmul(praw[:], lhsT=hidb[:], rhs=w2b[:], start=False, stop=True)
        raw_sb = spool.tile([SPC, MLP_OUT], f32)
        nc.scalar.activation(raw_sb[:], praw[:], func=AF.Identity, scale=WSCALE)

        # conv1 stationary 3x [36, 32] bf16: row dy*36 + s*9 + ci*3 + dx,
        # col s*8 + t = wt[s][t, ci, dy, dx] * 0.1/27. raw col q =
        # ci*72+dydx*8+t. Scatter through DRAM (host-zeroed) to keep SBUF
        # writes aligned.
        for dy in range(K):
            for dx in range(K):
                nc.sync.dma_start(
                    AP(tensor=lhsT1z, offset=(dy * 36 + dx) * 32,
                       ap=[[9 * 32 + 8, SPC], [3 * 32, CIN], [1, TMP]]),
                    AP(tensor=raw_sb.tensor, offset=(dy * 3 + dx) * TMP,
                       ap=[[MLP_OUT, SPC], [72, CIN], [1, TMP]]),
                )
        lhsT1 = []
        lhsT1b = []
        for dy in range(K):
            t_ = cpool.tile([36, SPC * TMP], bf16, name=f"lhsT1_{dy}")
            nc.gpsimd.dma_start(t_[:], lhsT1z.ap()[dy * 36:(dy + 1) * 36, :])
            lhsT1.append(t_)
            # same stationary duplicated on partitions 64..99 so band1's
            # LDWEIGHTS/rhs use PE rows 64..99 (no row-group conflict with
            # band0's in-flight matmuls)
            tb_ = cpool.tile([100, SPC * TMP], bf16, name=f"lhsT1b_{dy}")
            nc.gpsimd.dma_start(tb_[64:100, :],
                                lhsT1z.ap()[dy * 36:(dy + 1) * 36, :])
            lhsT1b.append(tb_)

        # conv1 bias [32,1] via DRAM staging: bias_d[s*8+t] = raw[s, 216+t]
        nc.sync.dma_start(
            AP(tensor=bias_d, offset=0, ap=[[TMP, SPC], [1, TMP]]),
            AP(tensor=raw_sb.tensor, offset=216, ap=[[MLP_OUT, SPC], [1, TMP]]),
        )
        bias1 = cpool.tile([2 * SPC * TMP, 1], f32)
        nc.sync.dma_start(bias1[0:SPC * TMP, :], bias_d.ap().unsqueeze(1))
        nc.sync.dma_start(bias1[SPC * TMP:2 * SPC * TMP, :],
                          bias_d.ap().unsqueeze(1))
        nc.vector.tensor_scalar_mul(bias1[:], bias1[:], float(K27))  # undo /27

        mlp_ctx.close()  # release MLP PSUM banks for conv pools

        # ============ conv pipeline pools ============
        xbp = ctx.enter_context(tc.tile_pool(name="xbp", bufs=2))
        ytp = ctx.enter_context(tc.tile_pool(name="ytp",
                                             bufs=4 if phased else 3))
        y4p = ctx.enter_context(tc.tile_pool(name="y4p", bufs=2))
        op_ = ctx.enter_context(tc.tile_pool(name="opool", bufs=2))
        psp = ctx.enter_context(tc.tile_pool(name="psp", bufs=4, space="PSUM"))

        ctr = [0]
        ybt_tiles = {}

        def conv1_pair(G):
            # produces ybt2(G): partitions (bp, s, t) hold band 2G+bp --
            # padded y rows p in [32(2G+bp), +35] at tile rows 0..35
            # (y row = p-1; rows 34,35 are overshoot). The two bands run
            # as concurrent PE column-tile groups.
            bands = [b for b in (2 * G, 2 * G + 1) if b < NB32]
            nbp = len(bands)
            # clean (non-replicated) DRAM read of the 38-row X bands:
            # partition (s,ci,dx) -- dx pre-shifted on host; plane row
            # 32b+k = X_pad row 32b-1+k (extra lead row on host). band1's
            # copy lives on partitions 64..99 so its PE row group is
            # disjoint from band0's.
            xb = xbp.tile([(nbp - 1) * 64 + 3 * SPC * CIN, XB4], bf16,
                          name=f"xb_{ctr[0]}_{G}", tag="xb")
            for bp, b in enumerate(bands):
                xeng = nc.scalar if (xb_split and bp == 1) else nc.sync
                xeng.dma_start(
                    xb[bp * 64:bp * 64 + 3 * SPC * CIN],
                    AP(tensor=padXb3, offset=b * 32 * PW,
                       ap=[[PP4, 3 * SPC * CIN], [1, XB4]]),
                )
            if variant == "conv1dma":
                return
            ybt = ytp.tile([2 * SPC * TMP, YBTS], bf16,
                           name=f"ybt_{ctr[0]}_{G}", tag="ybt")
            for b in bands:
                ybt_tiles[b] = ybt
            # K=36 contraction over (s,ci,dx); dy via 3 PSUM-accumulated
            # matmuls at row-shifted rhs offsets. y row (32b-1)+4j+2u+{0,1}
            # reads X_pad rows r..r+2 = xb rows 4j+2u+dy .. +1
            for j in range(9):
                p1 = psp.tile([128, 1024], f32,
                              name=f"p1_{ctr[0]}_{G}_{j}", tag="p2")
                # (bp, u) groups staggered so concurrent col-groups touch
                # different PSUM banks (bank-level has_written semantics)
                if nbp == 2:
                    gsets = [[(0, 0), (1, 1)], [(1, 0), (0, 1)]]
                else:
                    gsets = [[(0, 0)], [(0, 1)]]
                for gset in gsets:
                    for dy in range(K):
                        for bp, u in gset:
                            lw = lhsT1[dy][:] if bp == 0 else \
                                lhsT1b[dy][64:100, :]
                            nc.tensor.matmul(
                                p1[bp * 32:bp * 32 + SPC * TMP,
                                   u * 512:u * 512 + 448],
                                lhsT=lw,
                                rhs=AP(tensor=xb.tensor,
                                       offset=bp * 64 * XB4 +
                                       (4 * j + 2 * u + dy) * PW,
                                       ap=[[XB4, 3 * SPC * CIN],
                                           [PW, 2], [1, W]]),
                                start=(dy == 0), stop=(dy == 2),
                                tile_position=(bp * 64, bp * 32),
                            )
                # write ybt rows 4j..4j+3, interior cols (both bands)
                dst = AP(
                    tensor=ybt.tensor,
                    offset=4 * j * PW + 1,
                    ap=[[YBTS, nbp * SPC * TMP], [2 * PW, 2], [PW, 2], [1, W]],
                )
                pv = AP(
                    tensor=p1.tensor, offset=0,
                    ap=[[1024, nbp * SPC * TMP], [512, 2], [W, 2], [1, W]],
                )
                if variant == "conv1mm":
                    continue
                if j % 2 == 0:
                    nc.scalar.activation(dst, pv, func=AF.Identity,
                                         bias=bias1[0:nbp * SPC * TMP, :])
                else:
                    nc.vector.tensor_scalar_add(dst, pv,
                                                bias1[0:nbp * SPC * TMP, :])
            if variant == "conv1mm":
                return
            # zero the pad columns: (row i, col 225)+(row i+1, col 0) pairs,
            # plus (row 0, col 0)
            nc.vector.memset(ybt[:, 0:1], 0.0)
            nc.vector.memset(
                AP(tensor=ybt.tensor, offset=225,
                   ap=[[YBTS, nbp * SPC * TMP], [PW, 36], [1, 2]]), 0.0)
            if G == 0:
                # padded top row (y row -1) must be zero (band 0 = bp 0)
                nc.vector.memset(ybt[0:SPC * TMP, 0:PW], 0.0)
            if bands[-1] == NB32 - 1:
                # padded bottom row (tile row 33 = padded row 225) zero
                bp = nbp - 1
                nc.vector.memset(
                    ybt[bp * 32:bp * 32 + SPC * TMP, 33 * PW:34 * PW], 0.0)

        def conv2_band(gg):
            # yb[(s,t,dy'), e] = ybt(gg)[(s,t), dy'*PW + e]; one DMA:
            # dst partitions (s,t,dy') contiguous, dy' as a src row shift
            yb = y4p.tile([128, YB2], bf16, name=f"yb_{ctr[0]}_{gg}", tag="yb")
            ybt = ybt_tiles[gg]
            ln = 30 * PW + PW  # exact conv2 rhs read extent per partition
            if yb_eng == "split":
                # two halves (samples 01 / 23) on the two HWDGE rings so the
                # streams drain in parallel
                for hf, eng in ((0, nc.sync), (1, nc.scalar)):
                    eng.dma_start(
                        AP(tensor=yb.tensor, offset=hf * 64 * YB2,
                           ap=[[YB2, 64], [1, ln]]),
                        AP(tensor=ybt.tensor,
                           offset=((gg & 1) * 32 + hf * 16) * YBTS,
                           ap=[[YBTS, SPC * TMP // 2], [PW, 4], [1, ln]]),
                    )
            else:
                getattr(nc, yb_eng).dma_start(
                    AP(tensor=yb.tensor, offset=0, ap=[[YB2, 128], [1, ln]]),
                    AP(tensor=ybt.tensor, offset=(gg & 1) * 32 * YBTS,
                       ap=[[YBTS, SPC * TMP], [PW, 4], [1, ln]]),
                )
            if variant == "ybonly":
                return
            # bf16 staging: col s*3584 + q*224 + c holds
            # out[pix = partition>>6, s, :, 16gg + q, c]
            osb = None
            if variant != "conv2mm":
                osb = op_.tile([128, SPC * 3584], bf16,
                               name=f"o_{ctr[0]}_{gg}", tag="o")
            # 32 output rows (32gg..32gg+31) for all samples; sample pairs in
            # lockstep so each LDWEIGHTS overlaps the other row-group's MMs
            for sp in range(2):
                for k in range(4):          # 8-row psum tiles within the band
                    pt = []
                    for si in range(2):
                        pt.append(psp.tile(
                            [128, 1024], f32,
                            name=f"p2_{ctr[0]}_{gg}_{sp}_{k}_{si}", tag="p2"))
                    for dx in range(3):
                        for si in range(2):
                            s = sp * 2 + si
                            lw = lhsTc[s * 32:(s + 1) * 32,
                                       dx * 128:(dx + 1) * 128]
                            for bk in range(2):
                                Rl = 8 * k + 4 * bk
                                nc.tensor.matmul(
                                    pt[si][:, bk * 512:bk * 512 + 448], lhsT=lw,
                                    rhs=AP(tensor=yb.tensor,
                                           offset=(s * 32) * YB2 + Rl * PW + dx,
                                           ap=[[YB2, 32], [2 * PW, 2], [1, W]]),
                                    start=(dx == 0), stop=(dx == 2),
                                    tile_position=(s * 32, 0),
                                )
                    if variant == "conv2mm":
                        continue
                    for si in range(2):
                        s = sp * 2 + si
                        pv = AP(tensor=pt[si].tensor, offset=0,
                                ap=[[1024, 128], [512, 2], [W, 2], [1, W]])
                        ov = AP(tensor=osb.tensor, offset=s * 3584 + k * 896,
                                ap=[[SPC * 3584, 128], [448, 2], [W, 2], [1, W]])
                        if (s + k) % 2 == 0:
                            nc.scalar.activation(ov, pv, func=AF.Relu,
                                                 bias=cnnb_sb[:])
                        else:
                            nc.vector.tensor_scalar(ov, pv, cnnb_sb[:], 0.0,
                                                    op0=ALU.add, op1=ALU.max)
                if store_split and variant not in ("conv2mm", "nostores"):
                    # store this sample pair as soon as its epilogue is done
                    for pix in range(2):
                        nc.gpsimd.dma_start(
                            AP(tensor=out,
                               offset=pix * SPC * COUT * HB +
                               2 * sp * COUT * HB + gg * 16 * W,
                               ap=[[HB, COUT], [COUT * HB, 2], [1, 16 * W]]),
                            AP(tensor=osb.tensor,
                               offset=pix * 64 * SPC * 3584 + 2 * sp * 3584,
                               ap=[[SPC * 3584, 64], [3584, 2], [1, 3584]]),
                        )
            if variant in ("conv2mm", "nostores"):
                return
            # store the band: DMAs per pix block; dst [pix, s, c, 112, 224]
            # with 16 contiguous rows (7168B) per (s, c); SWDGE on Pool
            seng = getattr(nc, store_eng) if store_eng != "mix" else nc.gpsimd
            if not store_split:
                for pix in range(2):
                    seng.dma_start(
                        AP(tensor=out,
                           offset=pix * SPC * COUT * HB + gg * 16 * W,
                           ap=[[HB, COUT], [COUT * HB, SPC], [1, 16 * W]]),
                        AP(tensor=osb.tensor, offset=pix * 64 * SPC * 3584,
                           ap=[[SPC * 3584, 64], [3584, SPC], [1, 3584]]),
                    )

        NPAIR = (NB32 + 1) // 2

        def pipeline():
            if variant in ("conv1", "conv1dma", "conv1mm"):
                for G in range(NPAIR):
                    conv1_pair(G)
                ctr[0] += 1
                return
            if phased:
                for G in range(NPAIR):
                    conv1_pair(G)
                for gg in range(NB32):
                    conv2_band(gg)
            else:
                conv1_pair(0)
                for G in range(1, NPAIR):
                    conv1_pair(G)
                    conv2_band(2 * G - 2)
                    conv2_band(2 * G - 1)
                for gg in range(2 * NPAIR - 2, NB32):
                    conv2_band(gg)
            ctr[0] += 1

        if loop_n is not None:
            hints = [mybir.EngineType.PE, mybir.EngineType.Activation,
                     mybir.EngineType.DVE, mybir.EngineType.SP,
                     mybir.EngineType.Pool]
            with tc.For_i(0, loop_n, 1, hint_engines=hints):
                for _inner in range(inner):
                    pipeline()
        else:
            for _rep in range(repeat):
                pipeline()

    nc.compile()
    _CACHE[key] = nc
    return nc


def make_in_maps(X, flat_x, W1, b1, W2, b2, cnn_w, cnn_b):
    X = np.asarray(X, np.float32)
    flat_x = np.asarray(flat_x, np.float32)
    W1 = np.asarray(W1, np.float32)
    b1 = np.asarray(b1, np.float32)
    W2 = np.asarray(W2, np.float32)
    b2 = np.asarray(b2, np.float32)
    cnn_w = np.asarray(cnn_w, np.float32)
    cnn_b = np.asarray(cnn_b, np.float32)

    # plane rows: 0 = extra lead zero, 1..226 = X_pad rows 0..225 (X at
    # rows 2..225, cols 1..224), 227..229 = zero tail
    img = np.zeros((B, CIN, 230, PW), np.float32)
    img[:, :, 2:2 + H, 1:1 + W] = X
    Xpb = img.reshape(B, CIN, PP4).astype(ml_dtypes.bfloat16)
    fxT_full = np.ascontiguousarray(flat_x.T)                  # [128, 32]

    # W2 columns permuted: new col q = ci*72 + (dy*3+dx)*8 + t <- old
    # t*27 + ci*9 + dy*3 + dx (bias cols 216..223 unpermuted); b2 appended.
    perm = np.arange(MLP_OUT)
    for t in range(TMP):
        for ci in range(CIN):
            for dydx in range(9):
                perm[ci * 72 + dydx * 8 + t] = t * 27 + ci * 9 + dydx
    W2P = np.zeros((MLP_OUT + 1, MLP_OUT), np.float32)
    W2P[:MLP_OUT, :] = W2[:, perm]
    W2P[MLP_OUT, :] = b2[perm]
    lhsT1z = np.zeros((K108, SPC * TMP), np.float32)

    # conv2 stationary with vertical pixel-pairing:
    # base[t*4+dy', dx, pix*64+co] = cnn_w[co, t, dy'-pix, dx] (valid dy'-pix)
    base = np.zeros((32, 3, 128), np.float32)
    for dyp in range(4):
        for pix in range(2):
            dy = dyp - pix
            if 0 <= dy <= 2:
                for t in range(TMP):
                    base[t * 4 + dyp, :, pix * 64:(pix + 1) * 64] = \
                        cnn_w[:, t, dy, :].T
    cnn_wP = np.tile(base.reshape(32, 3 * 128), (4, 1))        # [128, 384]
    cnn_b128 = np.tile(cnn_b, 2)                               # [128]

    in_maps = []
    for i in range(NCORES):
        sl = slice(i * SPC, (i + 1) * SPC)
        src = Xpb[sl].reshape(SPC * CIN, PP4)
        padx_i = np.zeros((3 * SPC * CIN, PP4), ml_dtypes.bfloat16)
        for dx in range(3):
            padx_i[dx::3, :PP4 - dx] = src[:, dx:]
        in_maps.append({
            "padXb3": padx_i,
            "fxT": np.ascontiguousarray(fxT_full[:, sl]),
            "W1": W1, "b1": b1, "W2P": W2P, "lhsT1z": lhsT1z,
            "cnn_wP": cnn_wP, "cnn_b128": cnn_b128,
        })
    return in_maps


def kernel(X, flat_x, W1, b1, W2, b2, cnn_w, cnn_b):
    nc = build_module()
    in_maps = make_in_maps(X, flat_x, W1, b1, W2, b2, cnn_w, cnn_b)
    res = run_bass_kernel_spmd(nc, in_maps, core_ids=list(range(NCORES)))
    outs = []
    for i in range(NCORES):
        blk = np.asarray(res.results[i]["out"]).astype(np.float32).reshape(
            2, SPC, COUT, H // 2, W)
        o = np.empty((SPC, COUT, H, W), np.float32)
        o[:, :, 0::2] = blk[0]
        o[:, :, 1::2] = blk[1]
        outs.append(o)
    return np.concatenate(outs, axis=0)


# revision 76
# speedup vs baseline: 1.0218x; 1.0218x over previous
"""Trainium2 Bass kernel for nn_MergeMetaCNN (hypernetwork MLP -> grouped conv -> CNN).

Data-parallel over batch: 32 samples -> 8 NeuronCores, 4 samples each.

Per-core pipeline (all math on device):
  1. MLP (fp32 matmuls) -> per-sample conv1 filters + biases (as raw^T in DRAM).
  2. conv1 (grouped 3x3) as block-diag matmul: stationary [108, 32] bf16
     (rows = (tap, sample, cin)), moving operand = replicated/shifted bf16 X
     band tiles [108, 34*226] built by ONE DMA per band from the padded
     bf16 X planes (each partition = one (dy, dx, plane) shifted window).
     Epilogue (ACT/DVE alternating) writes y rows into padded y planes.
  3. y replicas: one DMA per band copies the y planes into Y4
     [128 = (s, t, dy'), PLANE] with per-partition row shift dy' in {0..3}.
  4. conv2 (8->64, 3x3) with vertical pixel-pairing: M = 128 = (pix in {0,1},
     cout); contraction K = 32 = (t, dy') per sample (PE row-group s via
     tile_position); the 3 kernel columns accumulate in PSUM as 3 matmuls
     whose rhs APs read Y4 at free-dim offsets dx in {0,1,2}.
  5. Epilogue relu(x + b) split across ScalarE/VectorE into bf16 staging;
     one SWDGE store DMA per band writes an even/odd row-block layout
     (contiguous 7KB runs); host un-interleaves + upcasts to fp32.
  conv1 band g+2 / conv2 band g are interleaved in program order so the
  xr gather, conv1 MM+epilogue, yb replication, conv2 MM+epilogue and the
  store all pipeline across bands.
"""

import numpy as np
import ml_dtypes
from contextlib import ExitStack

import concourse.bass as bass
import concourse.tile as tile
from concourse import bacc, mybir
from concourse.bass_utils import run_bass_kernel_spmd

AP = bass.AP
f32 = mybir.dt.float32
bf16 = mybir.dt.bfloat16
AF = mybir.ActivationFunctionType
ALU = mybir.AluOpType

# Problem constants (hardcoded per contract)
B, CIN, H, W = 32, 3, 224, 224
TMP, K, FLAT, COUT = 8, 3, 128, 64
MLP_OUT = TMP * CIN * K * K + TMP  # 224
META = 0.1
NCORES = 8
SPC = B // NCORES                  # 4 samples per core
PH, PW = H + 2, W + 2              # 226 (zero-pad 1 on each side)
PLANE = PH * PW                    # 51076
PP4 = 230 * PW                     # X plane stride: 1 extra lead row + tail
K27 = CIN * K * K                  # 27
K108 = SPC * K27                   # conv1 contraction (block-diag 4 samples)
NB32 = H // 32                     # 7 bands of 32 rows
XB4 = 37 * PW + W                  # staged X band (exact conv1 read extent)
YBTS = 37 * PW                     # y band tile stride (36 rows + 1 overread)
YB2 = 34 * PW                      # conv2 y-replica band length
HWP = H * W
HB = (H // 2) * W                  # even/odd half-plane block (112*224)

_CACHE = {}


def build_module(repeat=1, loop_n=None, variant="full", store_eng="gpsimd",
                 inner=1, yb_eng="split", xb_split=False, store_split=True,
                 phased=False, xb_eng="sync"):
    key = ("nc", repeat, loop_n, variant, store_eng, inner, yb_eng,
           xb_split, store_split, phased, xb_eng)
    if key in _CACHE:
        return _CACHE[key]
    nc = bacc.Bacc("TRN2", target_bir_lowering=False, debug=False, num_devices=NCORES)

    # ---- DRAM I/O (per-core shapes) ----
    # padXb3: 36 planes (s, ci, dx) of bf16 padded X, pre-shifted by dx on
    # the host; each plane has 1 extra zero lead row + zero tail rows so
    # every band reads a uniform in-bounds window
    padXb3 = nc.dram_tensor("padXb3", [3 * SPC * CIN, PP4], bf16,
                            kind="ExternalInput")
    fxT = nc.dram_tensor("fxT", [FLAT, SPC], f32, kind="ExternalInput")
    W1 = nc.dram_tensor("W1", [FLAT, MLP_OUT], f32, kind="ExternalInput")
    b1 = nc.dram_tensor("b1", [MLP_OUT], f32, kind="ExternalInput")
    # W2P: columns 0..215 permuted to q = ci*72 + (dy*3+dx)*8 + t; row 224 = b2
    W2P = nc.dram_tensor("W2P", [MLP_OUT + 1, MLP_OUT], f32, kind="ExternalInput")
    # lhsT1z: zeros [108, 32] staging for the conv1-stationary scatter
    lhsT1z = nc.dram_tensor("lhsT1z", [K108, SPC * TMP], f32,
                            kind="ExternalInput")
    bias_d = nc.dram_tensor("bias_d", [SPC * TMP], f32)   # conv1 bias staging
    cnn_wP = nc.dram_tensor("cnn_wP", [128, 3 * 128], f32, kind="ExternalInput")
    cnn_b128 = nc.dram_tensor("cnn_b128", [128], f32, kind="ExternalInput")
    # timing builds (loop_n) keep the result in DRAM and return a tiny dummy
    # output so runs don't ship 25.7MB/core back through the axon tunnel
    timing = loop_n is not None
    # out layout: [pix, s, c, 112, 224] -- even rows block then odd rows
    # block (host un-interleaves); gives 7168B-contiguous store runs
    out = nc.dram_tensor("out", [2, SPC * COUT * HB], bf16,
                         kind="Internal" if timing else "ExternalOutput")
    if timing:
        dummy = nc.dram_tensor("tdummy", [1, 32], f32, kind="ExternalOutput")

    with tile.TileContext(nc) as tc, ExitStack() as ctx:
        cpool = ctx.enter_context(tc.tile_pool(name="consts", bufs=1))
        spool = ctx.enter_context(tc.tile_pool(name="stageA", bufs=1))
        mlp_ctx = ExitStack()
        mpsum = mlp_ctx.enter_context(tc.tile_pool(name="mlp_psum", bufs=2, space="PSUM"))

        # ================= Stage A: MLP + weight prep =================
        w1sb = cpool.tile([FLAT, MLP_OUT], f32)
        nc.sync.dma_start(w1sb[:], W1.ap())
        w2a = cpool.tile([128, MLP_OUT], f32)
        nc.sync.dma_start(w2a[:], W2P.ap()[0:128, :])
        w2b = cpool.tile([97, MLP_OUT], f32)          # rows 128..224 (incl. b2)
        nc.sync.dma_start(w2b[:], W2P.ap()[128:225, :])
        fx_sb = cpool.tile([FLAT, SPC], f32)
        nc.sync.dma_start(fx_sb[:], fxT.ap())
        b1a = cpool.tile([128, 1], f32)
        nc.sync.dma_start(b1a[:], b1.ap()[0:128].unsqueeze(1))
        b1b = cpool.tile([96, 1], f32)
        nc.sync.dma_start(b1b[:], b1.ap()[128:224].unsqueeze(1))
        cnnb_sb = cpool.tile([128, 1], f32)
        nc.sync.dma_start(cnnb_sb[:], cnn_b128.ap().unsqueeze(1))
        lhsTc = cpool.tile([128, 3 * 128], bf16)
        nc.gpsimd.dma_start(lhsTc[:], cnn_wP.ap())  # cast f32 -> bf16

        WSCALE = META / K27
        if timing:
            nc.sync.dma_start(dummy.ap(), b1.ap()[0:32].unsqueeze(0))

        # hid^T = relu(W1^T @ fxT + b1)  [224, SPC]; extra ones-row for b2
        ph_a = mpsum.tile([128, SPC], f32, tag="mp")
        nc.tensor.matmul(ph_a[:], lhsT=w1sb[:, 0:128], rhs=fx_sb[:], start=True, stop=True)
        hida = spool.tile([128, SPC], f32)
        nc.scalar.activation(hida[:], ph_a[:], func=AF.Relu, bias=b1a[:])
        ph_b = mpsum.tile([96, SPC], f32, tag="mp")
        nc.tensor.matmul(ph_b[:], lhsT=w1sb[:, 128:224], rhs=fx_sb[:], start=True, stop=True)
        hidb = spool.tile([97, SPC], f32)
        nc.vector.memset(hidb[96:97, :], 1.0)
        nc.scalar.activation(hidb[0:96, :], ph_b[:], func=AF.Relu, bias=b1b[:])

        # raw = (hid^T)^T @ W2P * WSCALE  [SPC, 224]  (sample-major, permuted)
        praw = mpsum.tile([SPC, MLP_OUT], f32, tag="mp")
        nc.tensor.matmul(praw[:], lhsT=hida[:], rhs=w2a[:], start=True, stop=False)
        nc.tensor.matmul(praw[:], lhsT=hidb[:], rhs=w2b[:], start=False, stop=True)
        raw_sb = spool.tile([SPC, MLP_OUT], f32)
        nc.scalar.activation(raw_sb[:], praw[:], func=AF.Identity, scale=WSCALE)

        # conv1 stationary 3x [36, 32] bf16: row dy*36 + s*9 + ci*3 + dx,
        # col s*8 + t = wt[s][t, ci, dy, dx] * 0.1/27. raw col q =
        # ci*72+dydx*8+t. Scatter through DRAM (host-zeroed) to keep SBUF
        # writes aligned.
        for dy in range(K):
            for dx in range(K):
                nc.sync.dma_start(
                    AP(tensor=lhsT1z, offset=(dy * 36 + dx) * 32,
                       ap=[[9 * 32 + 8, SPC], [3 * 32, CIN], [1, TMP]]),
                    AP(tensor=raw_sb.tensor, offset=(dy * 3 + dx) * TMP,
                       ap=[[MLP_OUT, SPC], [72, CIN], [1, TMP]]),
                )
        lhsT1 = []
        lhsT1b = []
        for dy in range(K):
            t_ = cpool.tile([36, SPC * TMP], bf16, name=f"lhsT1_{dy}")
            nc.gpsimd.dma_start(t_[:], lhsT1z.ap()[dy * 36:(dy + 1) * 36, :])
            lhsT1.append(t_)
            # same stationary duplicated on partitions 64..99 so band1's
            # LDWEIGHTS/rhs use PE rows 64..99 (no row-group conflict with
            # band0's in-flight matmuls)
            tb_ = cpool.tile([100, SPC * TMP], bf16, name=f"lhsT1b_{dy}")
            nc.gpsimd.dma_start(tb_[64:100, :],
                                lhsT1z.ap()[dy * 36:(dy + 1) * 36, :])
            lhsT1b.append(tb_)

        # conv1 bias [32,1] via DRAM staging: bias_d[s*8+t] = raw[s, 216+t]
        nc.sync.dma_start(
            AP(tensor=bias_d, offset=0, ap=[[TMP, SPC], [1, TMP]]),
            AP(tensor=raw_sb.tensor, offset=216, ap=[[MLP_OUT, SPC], [1, TMP]]),
        )
        bias1 = cpool.tile([2 * SPC * TMP, 1], f32)
        nc.sync.dma_start(bias1[0:SPC * TMP, :], bias_d.ap().unsqueeze(1))
        nc.sync.dma_start(bias1[SPC * TMP:2 * SPC * TMP, :],
                          bias_d.ap().unsqueeze(1))
        nc.vector.tensor_scalar_mul(bias1[:], bias1[:], float(K27))  # undo /27

        mlp_ctx.close()  # release MLP PSUM banks for conv pools

        # ============ conv pipeline pools ============
        xbp = ctx.enter_context(tc.tile_pool(name="xbp", bufs=2))
        ytp = ctx.enter_context(tc.tile_pool(name="ytp",
                                             bufs=4 if phased else 3))
        y4p = ctx.enter_context(tc.tile_pool(name="y4p", bufs=3))
        op_ = ctx.enter_context(tc.tile_pool(name="opool", bufs=2))
        psp = ctx.enter_context(tc.tile_pool(name="psp", bufs=4, space="PSUM"))

        ctr = [0]
        ybt_tiles = {}

        def conv1_pair(G):
            # produces ybt2(G): partitions (bp, s, t) hold band 2G+bp --
            # padded y rows p in [32(2G+bp), +35] at tile rows 0..35
            # (y row = p-1; rows 34,35 are overshoot). The two bands run
            # as concurrent PE column-tile groups.
            bands = [b for b in (2 * G, 2 * G + 1) if b < NB32]
            nbp = len(bands)
            # clean (non-replicated) DRAM read of the 38-row X bands:
            # partition (s,ci,dx) -- dx pre-shifted on host; plane row
            # 32b+k = X_pad row 32b-1+k (extra lead row on host). band1's
            # copy lives on partitions 64..99 so its PE row group is
            # disjoint from band0's.
            xb = xbp.tile([(nbp - 1) * 64 + 3 * SPC * CIN, XB4], bf16,
                          name=f"xb_{ctr[0]}_{G}", tag="xb")
            for bp, b in enumerate(bands):
                if xb_split:
                    xeng = nc.scalar if bp == 1 else nc.sync
                else:
                    xeng = getattr(nc, xb_eng)
                xeng.dma_start(
                    xb[bp * 64:bp * 64 + 3 * SPC * CIN],
                    AP(tensor=padXb3, offset=b * 32 * PW,
                       ap=[[PP4, 3 * SPC * CIN], [1, XB4]]),
                )
            if variant == "conv1dma":
                return
            ybt = ytp.tile([2 * SPC * TMP, YBTS], bf16,
                           name=f"ybt_{ctr[0]}_{G}", tag="ybt")
            for b in bands:
                ybt_tiles[b] = ybt
            # K=36 contraction over (s,ci,dx); dy via 3 PSUM-accumulated
            # matmuls at row-shifted rhs offsets. y row (32b-1)+4j+2u+{0,1}
            # reads X_pad rows r..r+2 = xb rows 4j+2u+dy .. +1
            for j in range(9):
                p1 = psp.tile([128, 1024], f32,
                              name=f"p1_{ctr[0]}_{G}_{j}", tag="p2")
                # (bp, u) groups staggered so concurrent col-groups touch
                # different PSUM banks (bank-level has_written semantics)
                if nbp == 2:
                    gsets = [[(0, 0), (1, 1)], [(1, 0), (0, 1)]]
                else:
                    gsets = [[(0, 0)], [(0, 1)]]
                for gset in gsets:
                    for dy in range(K):
                        for bp, u in gset:
                            lw = lhsT1[dy][:] if bp == 0 else \
                                lhsT1b[dy][64:100, :]
                            nc.tensor.matmul(
                                p1[bp * 32:bp * 32 + SPC * TMP,
                                   u * 512:u * 512 + 448],
                                lhsT=lw,
                                rhs=AP(tensor=xb.tensor,
                                       offset=bp * 64 * XB4 +
                                       (4 * j + 2 * u + dy) * PW,
                                       ap=[[XB4, 3 * SPC * CIN],
                                           [PW, 2], [1, W]]),
                                start=(dy == 0), stop=(dy == 2),
                                tile_position=(bp * 64, bp * 32),
                            )
                # write ybt rows 4j..4j+3, interior cols (both bands)
                dst = AP(
                    tensor=ybt.tensor,
                    offset=4 * j * PW + 1,
                    ap=[[YBTS, nbp * SPC * TMP], [2 * PW, 2], [PW, 2], [1, W]],
                )
                pv = AP(
                    tensor=p1.tensor, offset=0,
                    ap=[[1024, nbp * SPC * TMP], [512, 2], [W, 2], [1, W]],
                )
                if variant == "conv1mm":
                    continue
                if j % 2 == 0:
                    nc.scalar.activation(dst, pv, func=AF.Identity,
                                         bias=bias1[0:nbp * SPC * TMP, :])
                else:
                    nc.vector.tensor_scalar_add(dst, pv,
                                                bias1[0:nbp * SPC * TMP, :])
            if variant == "conv1mm":
                return
            # zero the pad columns: (row i, col 225)+(row i+1, col 0) pairs,
            # plus (row 0, col 0)
            nc.vector.memset(ybt[:, 0:1], 0.0)
            nc.vector.memset(
                AP(tensor=ybt.tensor, offset=225,
                   ap=[[YBTS, nbp * SPC * TMP], [PW, 36], [1, 2]]), 0.0)
            if G == 0:
                # padded top row (y row -1) must be zero (band 0 = bp 0)
                nc.vector.memset(ybt[0:SPC * TMP, 0:PW], 0.0)
            if bands[-1] == NB32 - 1:
                # padded bottom row (tile row 33 = padded row 225) zero
                bp = nbp - 1
                nc.vector.memset(
                    ybt[bp * 32:bp * 32 + SPC * TMP, 33 * PW:34 * PW], 0.0)

        def conv2_band(gg):
            # yb[(s,t,dy'), e] = ybt(gg)[(s,t), dy'*PW + e]; one DMA:
            # dst partitions (s,t,dy') contiguous, dy' as a src row shift
            yb = y4p.tile([128, YB2], bf16, name=f"yb_{ctr[0]}_{gg}", tag="yb")
            ybt = ybt_tiles[gg]
            ln = 30 * PW + PW  # exact conv2 rhs read extent per partition
            if yb_eng == "split":
                # two halves (samples 01 / 23) on the two HWDGE rings so the
                # streams drain in parallel
                for hf, eng in ((0, nc.sync), (1, nc.scalar)):
                    eng.dma_start(
                        AP(tensor=yb.tensor, offset=hf * 64 * YB2,
                           ap=[[YB2, 64], [1, ln]]),
                        AP(tensor=ybt.tensor,
                           offset=((gg & 1) * 32 + hf * 16) * YBTS,
                           ap=[[YBTS, SPC * TMP // 2], [PW, 4], [1, ln]]),
                    )
            else:
                getattr(nc, yb_eng).dma_start(
                    AP(tensor=yb.tensor, offset=0, ap=[[YB2, 128], [1, ln]]),
                    AP(tensor=ybt.tensor, offset=(gg & 1) * 32 * YBTS,
                       ap=[[YBTS, SPC * TMP], [PW, 4], [1, ln]]),
                )
            if variant == "ybonly":
                return
            # bf16 staging: col s*3584 + q*224 + c holds
            # out[pix = partition>>6, s, :, 16gg + q, c]
            osb = None
            if variant != "conv2mm":
                osb = op_.tile([128, SPC * 3584], bf16,
                               name=f"o_{ctr[0]}_{gg}", tag="o")
            # 32 output rows (32gg..32gg+31) for all samples; sample pairs in
            # lockstep so each LDWEIGHTS overlaps the other row-group's MMs
            for sp in range(2):
                for k in range(4):          # 8-row psum tiles within the band
                    pt = []
                    for si in range(2):
                        pt.append(psp.tile(
                            [128, 1024], f32,
                            name=f"p2_{ctr[0]}_{gg}_{sp}_{k}_{si}", tag="p2"))
                    for dx in range(3):
                        for si in range(2):
                            s = sp * 2 + si
                            lw = lhsTc[s * 32:(s + 1) * 32,
                                       dx * 128:(dx + 1) * 128]
                            for bk in range(2):
                                Rl = 8 * k + 4 * bk
                                nc.tensor.matmul(
                                    pt[si][:, bk * 512:bk * 512 + 448], lhsT=lw,
                                    rhs=AP(tensor=yb.tensor,
                                           offset=(s * 32) * YB2 + Rl * PW + dx,
                                           ap=[[YB2, 32], [2 * PW, 2], [1, W]]),
                                    start=(dx == 0), stop=(dx == 2),
                                    tile_position=(s * 32, 0),
                                )
                    if variant == "conv2mm":
                        continue
                    for si in range(2):
                        s = sp * 2 + si
                        pv = AP(tensor=pt[si].tensor, offset=0,
                                ap=[[1024, 128], [512, 2], [W, 2], [1, W]])
                        ov = AP(tensor=osb.tensor, offset=s * 3584 + k * 896,
                                ap=[[SPC * 3584, 128], [448, 2], [W, 2], [1, W]])
                        if (s + k) % 2 == 0:
                            nc.scalar.activation(ov, pv, func=AF.Relu,
                                                 bias=cnnb_sb[:])
                        else:
                            nc.vector.tensor_scalar(ov, pv, cnnb_sb[:], 0.0,
                                                    op0=ALU.add, op1=ALU.max)
                if store_split and variant not in ("conv2mm", "nostores"):
                    # store this sample pair as soon as its epilogue is done
                    for pix in range(2):
                        nc.gpsimd.dma_start(
                            AP(tensor=out,
                               offset=pix * SPC * COUT * HB +
                               2 * sp * COUT * HB + gg * 16 * W,
                               ap=[[HB, COUT], [COUT * HB, 2], [1, 16 * W]]),
                            AP(tensor=osb.tensor,
                               offset=pix * 64 * SPC * 3584 + 2 * sp * 3584,
                               ap=[[SPC * 3584, 64], [3584, 2], [1, 3584]]),
                        )
            if variant in ("conv2mm", "nostores"):
                return
            # store the band: DMAs per pix block; dst [pix, s, c, 112, 224]
            # with 16 contiguous rows (7168B) per (s, c); SWDGE on Pool
            seng = getattr(nc, store_eng) if store_eng != "mix" else nc.gpsimd
            if not store_split:
                for pix in range(2):
                    seng.dma_start(
                        AP(tensor=out,
                           offset=pix * SPC * COUT * HB + gg * 16 * W,
                           ap=[[HB, COUT], [COUT * HB, SPC], [1, 16 * W]]),
                        AP(tensor=osb.tensor, offset=pix * 64 * SPC * 3584,
                           ap=[[SPC * 3584, 64], [3584, SPC], [1, 3584]]),
                    )

        NPAIR = (NB32 + 1) // 2

        def pipeline():
            if variant in ("conv1", "conv1dma", "conv1mm"):
                for G in range(NPAIR):
                    conv1_pair(G)
                ctr[0] += 1
                return
            if phased:
                for G in range(NPAIR):
                    conv1_pair(G)
                for gg in range(NB32):
                    conv2_band(gg)
            else:
                conv1_pair(0)
                for G in range(1, NPAIR):
                    conv1_pair(G)
                    conv2_band(2 * G - 2)
                    conv2_band(2 * G - 1)
                for gg in range(2 * NPAIR - 2, NB32):
                    conv2_band(gg)
            ctr[0] += 1

        if loop_n is not None:
            hints = [mybir.EngineType.PE, mybir.EngineType.Activation,
                     mybir.EngineType.DVE, mybir.EngineType.SP,
                     mybir.EngineType.Pool]
            with tc.For_i(0, loop_n, 1, hint_engines=hints):
                for _inner in range(inner):
                    pipeline()
        else:
            for _rep in range(repeat):
                pipeline()

    nc.compile()
    _CACHE[key] = nc
    return nc


def make_in_maps(X, flat_x, W1, b1, W2, b2, cnn_w, cnn_b):
    X = np.asarray(X, np.float32)
    flat_x = np.asarray(flat_x, np.float32)
    W1 = np.asarray(W1, np.float32)
    b1 = np.asarray(b1, np.float32)
    W2 = np.asarray(W2, np.float32)
    b2 = np.asarray(b2, np.float32)
    cnn_w = np.asarray(cnn_w, np.float32)
    cnn_b = np.asarray(cnn_b, np.float32)

    # plane rows: 0 = extra lead zero, 1..226 = X_pad rows 0..225 (X at
    # rows 2..225, cols 1..224), 227..229 = zero tail
    img = np.zeros((B, CIN, 230, PW), np.float32)
    img[:, :, 2:2 + H, 1:1 + W] = X
    Xpb = img.reshape(B, CIN, PP4).astype(ml_dtypes.bfloat16)
    fxT_full = np.ascontiguousarray(flat_x.T)                  # [128, 32]

    # W2 columns permuted: new col q = ci*72 + (dy*3+dx)*8 + t <- old
    # t*27 + ci*9 + dy*3 + dx (bias cols 216..223 unpermuted); b2 appended.
    perm = np.arange(MLP_OUT)
    for t in range(TMP):
        for ci in range(CIN):
            for dydx in range(9):
                perm[ci * 72 + dydx * 8 + t] = t * 27 + ci * 9 + dydx
    W2P = np.zeros((MLP_OUT + 1, MLP_OUT), np.float32)
    W2P[:MLP_OUT, :] = W2[:, perm]
    W2P[MLP_OUT, :] = b2[perm]
    lhsT1z = np.zeros((K108, SPC * TMP), np.float32)

    # conv2 stationary with vertical pixel-pairing:
    # base[t*4+dy', dx, pix*64+co] = cnn_w[co, t, dy'-pix, dx] (valid dy'-pix)
    base = np.zeros((32, 3, 128), np.float32)
    for dyp in range(4):
        for pix in range(2):
            dy = dyp - pix
            if 0 <= dy <= 2:
                for t in range(TMP):
                    base[t * 4 + dyp, :, pix * 64:(pix + 1) * 64] = \
                        cnn_w[:, t, dy, :].T
    cnn_wP = np.tile(base.reshape(32, 3 * 128), (4, 1))        # [128, 384]
    cnn_b128 = np.tile(cnn_b, 2)                               # [128]

    in_maps = []
    for i in range(NCORES):
        sl = slice(i * SPC, (i + 1) * SPC)
        src = Xpb[sl].reshape(SPC * CIN, PP4)
        padx_i = np.zeros((3 * SPC * CIN, PP4), ml_dtypes.bfloat16)
        for dx in range(3):
            padx_i[dx::3, :PP4 - dx] = src[:, dx:]
        in_maps.append({
            "padXb3": padx_i,
            "fxT": np.ascontiguousarray(fxT_full[:, sl]),
            "W1": W1, "b1": b1, "W2P": W2P, "lhsT1z": lhsT1z,
            "cnn_wP": cnn_wP, "cnn_b128": cnn_b128,
        })
    return in_maps


def kernel(X, flat_x, W1, b1, W2, b2, cnn_w, cnn_b):
    nc = build_module()
    in_maps = make_in_maps(X, flat_x, W1, b1, W2, b2, cnn_w, cnn_b)
    res = run_bass_kernel_spmd(nc, in_maps, core_ids=list(range(NCORES)))
    outs = []
    for i in range(NCORES):
        blk = np.asarray(res.results[i]["out"]).astype(np.float32).reshape(
            2, SPC, COUT, H // 2, W)
        o = np.empty((SPC, COUT, H, W), np.float32)
        o[:, :, 0::2] = blk[0]
        o[:, :, 1::2] = blk[1]
        outs.append(o)
    return np.concatenate(outs, axis=0)
